# Initial kernel scaffold
#
"""CrossAttention kernel for 8 TRN2 NeuronCores.

Reference computation (B=2, Lq=4096, Lkv=1024, query_dim=512, cross_dim=768,
heads=8, dim_head=64, inner=512):
    q = hs @ Wq; k = enc @ Wk; v = enc @ Wv          (per batch)
    attn = softmax(q_h @ k_h^T * scale) per head
    out = concat_h(attn @ v_h) @ Wo + bo

Sharding: 8 cores = 2 batches x 4 query-slices of 1024 queries.  Each core
computes its full slice of the output (all heads), so outputs are disjoint
and no collective is needed.

Per-core dataflow (all matmuls in float32r at full PE rate):
  - host passes hs-slice and encoder transposed (hsT [512,1024], encT [768,1024])
  - qT = Wq^T-weighted hsT   -> [inner=512, q=1024]  (heads along partitions)
  - kT likewise              -> [inner=512, kv=1024]
  - v natural                -> [kv=1024, slots]  slot h = 128 cols holding
        v_h (64) + a ones column + zero padding, arranged so the AV matmul
        output lands partition-aligned with head h's rows of outT and the
        softmax denominator (sum_kv exp) falls out of the same matmul.
  - scoresT_h = k_h qT_h     -> [kv, q] (kv on partitions; head pairs packed
        into the 128x128 PE array via base-partition row tiling)
  - expT = exp(scale * scoresT) on ScalarE (no max-subtraction: |scores|<~2)
  - outT_unnorm_h = v_slot^T @ expT accumulated over kv chunks (PSUM),
        one row of which is the softmax denominator
  - normalize: reciprocal (DVE) + partition_broadcast (GPSIMD) + multiply
  - final = outT^T @ Wo + bo -> [1024, 512], DMA out
"""

import sys

if "/opt/trn_rl_repo" not in sys.path:
    sys.path.insert(0, "/opt/trn_rl_repo")

import numpy as np

B, LQ, LKV = 2, 4096, 1024
QD, CD = 512, 768
H, DH = 8, 64
INNER = H * DH  # 512
SCALE = DH ** -0.5
NCORES = 8
QSH = LQ // 4  # 1024 queries per core
P = 128

_CACHE: dict = {}
LAST_RESULTS = None  # test harness introspection (exec_time_ns etc.)


def _build_nc():
    from contextlib import ExitStack

    import concourse.bass as bass
    import concourse.tile as tile
    from concourse import bacc, mybir

    f32 = mybir.dt.float32
    f32r = mybir.dt.float32r
    Exp = mybir.ActivationFunctionType.Exp

    nc = bacc.Bacc(trn_type="TRN2")

    hsT_d = nc.declare_dram_parameter("hsT", [QD, QSH], f32r, isOutput=False)
    encT_d = nc.declare_dram_parameter("encT", [CD, LKV], f32r, isOutput=False)
    wq_d = nc.declare_dram_parameter("wq", [QD, INNER], f32r, isOutput=False)
    wk_d = nc.declare_dram_parameter("wk", [CD, INNER], f32r, isOutput=False)
    wv_d = nc.declare_dram_parameter("wv", [CD, INNER], f32r, isOutput=False)
    wo_d = nc.declare_dram_parameter("wo", [INNER, QD], f32r, isOutput=False)
    bo_d = nc.declare_dram_parameter("bo", [1, QD], f32, isOutput=False)
    out_d = nc.declare_dram_parameter("out", [QSH, QD], f32, isOutput=True)

    KC_Q = QD // P   # 4 contraction chunks for q projection
    KC_KV = CD // P  # 6 for k/v projections
    AT = INNER // P  # 4 inner tiles (2 heads each)
    NT = LKV // P    # 8 kv chunks
    QN = QSH // 512  # 2 q slices of 512

    def r(ap):
        return ap

    with ExitStack() as ctx:
        tc = ctx.enter_context(tile.TileContext(nc))
        const = ctx.enter_context(tc.tile_pool(name="const", bufs=1))
        acts = ctx.enter_context(tc.tile_pool(name="acts", bufs=1))
        expp = ctx.enter_context(tc.tile_pool(name="expp", bufs=3))
        outp = ctx.enter_context(tc.tile_pool(name="outp", bufs=2))
        small = ctx.enter_context(tc.tile_pool(name="small", bufs=4))
        psA = ctx.enter_context(tc.tile_pool(name="psA", bufs=4, space="PSUM"))
        psS = ctx.enter_context(tc.tile_pool(name="psS", bufs=2, space="PSUM"))
        drp = ctx.enter_context(tc.tile_pool(name="drp", bufs=4, space="DRAM"))

        # ---- input DMA
        wq_sb = const.tile([P, KC_Q, INNER], f32r)
        nc.sync.dma_start(wq_sb[:], wq_d.rearrange("(c p) n -> p c n", p=P))
        wk_sb = const.tile([P, KC_KV, INNER], f32r)
        nc.sync.dma_start(wk_sb[:], wk_d.rearrange("(c p) n -> p c n", p=P))
        wv_sb = const.tile([P, KC_KV, INNER], f32r)
        nc.sync.dma_start(wv_sb[:], wv_d.rearrange("(c p) n -> p c n", p=P))
        wo_sb = const.tile([P, AT, QD], f32r)
        nc.sync.dma_start(wo_sb[:], wo_d.rearrange("(c p) n -> p c n", p=P))
        bo_sb = const.tile([P, QD], f32)
        nc.sync.dma_start(bo_sb[:], bo_d.ap().to_broadcast((P, QD)))
        hsT_sb = acts.tile([P, KC_Q, QSH], f32r)
        nc.sync.dma_start(hsT_sb[:], hsT_d.rearrange("(c p) n -> p c n", p=P))
        encT_sb = acts.tile([P, KC_KV, LKV], f32r)
        nc.sync.dma_start(encT_sb[:], encT_d.rearrange("(c p) n -> p c n", p=P))

        # ---- projections
        # qT [inner, q]: lhsT = Wq chunk, rhs = hsT chunk
        qT_sb = acts.tile([P, AT, QSH], f32r)
        for a in range(AT):
            for n in range(QN):
                ps = psA.tile([P, 512], f32, tag="acc")
                for c in range(KC_Q):
                    nc.tensor.matmul(
                        ps[:],
                        r(wq_sb[:, c, a * P:(a + 1) * P]),
                        r(hsT_sb[:, c, n * 512:(n + 1) * 512]),
                        start=(c == 0),
                        stop=(c == KC_Q - 1),
                    )
                nc.scalar.copy(qT_sb[:, a, n * 512:(n + 1) * 512], ps[:])

        # kT [inner, kv]
        kT_sb = acts.tile([P, AT, LKV], f32r)
        for a in range(AT):
            for n in range(LKV // 512):
                ps = psA.tile([P, 512], f32, tag="acc")
                for c in range(KC_KV):
                    nc.tensor.matmul(
                        ps[:],
                        r(wk_sb[:, c, a * P:(a + 1) * P]),
                        r(encT_sb[:, c, n * 512:(n + 1) * 512]),
                        start=(c == 0),
                        stop=(c == KC_KV - 1),
                    )
                nc.scalar.copy(kT_sb[:, a, n * 512:(n + 1) * 512], ps[:])

        # v natural [kv, slots]: slot h (128 wide):
        #   h even: [v_h (0:64) | 1.0 at 64 | 0 at 65:128]   -> out rows 0:64, denom row 64
        #   h odd : [1.0 at 0 | 0 at 1:64 | v_h at 64:128]   -> out rows 64:128, denom row 0
        # pad columns stay uninitialized: each psum output row reads only its
        # own lhsT column, and rows fed by pad columns are never read.
        v_sb = acts.tile([P, NT, H * P], f32r)
        vv4 = v_sb.rearrange("p t (s c) -> p t s c", c=P)
        for t in range(NT):
            nc.vector.memset(vv4[:, t, 0::2, 64:65].bitcast(f32), 1.0)
            nc.vector.memset(vv4[:, t, 1::2, 0:1].bitcast(f32), 1.0)
        for t in range(NT):
            ps = psA.tile([P, 512], f32, tag="acc")
            for c in range(KC_KV):
                nc.tensor.matmul(
                    ps[:],
                    r(encT_sb[:, c, t * P:(t + 1) * P]),
                    r(wv_sb[:, c, :]),
                    start=(c == 0),
                    stop=(c == KC_KV - 1),
                )
            vv = v_sb.rearrange("p t (s c) -> p t s c", c=P)
            pv = ps.rearrange("p (s c) -> p s c", c=DH)
            nc.vector.tensor_copy(vv[:, t, 0::2, 0:DH], pv[:, 0::2, :])
            nc.vector.tensor_copy(vv[:, t, 1::2, DH:P], pv[:, 1::2, :])

        # ---- attention + output accumulation
        outT_sb = acts.tile([P, AT, QSH], f32r)
        for hp in range(AT):  # head pair
            for n in range(QN):
                av0 = psA.tile([P, 512], f32, tag="acc")
                av1 = psA.tile([P, 512], f32, tag="acc")
                av = (av0, av1)
                for t in range(NT):
                    ss = psS.tile([P, 1024], f32)
                    for i in range(2):
                        pr = slice(i * 64, (i + 1) * 64)
                        nc.tensor.matmul(
                            ss[:, i * 512:(i + 1) * 512],
                            r(kT_sb[pr, hp, t * P:(t + 1) * P]),
                            r(qT_sb[pr, hp, n * 512:(n + 1) * 512]),
                            start=True,
                            stop=True,
                        )
                    ex = expp.tile([P, 1024], f32r)
                    nc.scalar.activation(ex[:], ss[:], Exp, scale=SCALE)
                    for i in range(2):
                        s = 2 * hp + i
                        nc.tensor.matmul(
                            av[i][:],
                            r(v_sb[:, t, s * P:(s + 1) * P]),
                            r(ex[:, i * 512:(i + 1) * 512]),
                            start=(t == 0),
                            stop=(t == NT - 1),
                        )
                # normalize and place into outT
                for i in range(2):
                    drow = 64 if i == 0 else 0
                    dst = slice(0, 64) if i == 0 else slice(64, 128)
                    rc = small.tile([P, 512], f32, tag="rc")
                    nc.vector.reciprocal(
                        rc[drow:drow + 1, :], av[i][drow:drow + 1, :]
                    )
                    dscr = drp.tile([1, 512], f32, tag="dscr")
                    nc.sync.dma_start(dscr[:], rc[drow:drow + 1, :])
                    rcb = small.tile([P, 512], f32, tag="rcb")
                    nc.sync.dma_start(
                        rcb[dst, :], dscr[:].to_broadcast((64, 512))
                    )
                    nc.vector.tensor_mul(
                        outT_sb[dst, hp, n * 512:(n + 1) * 512],
                        av[i][dst, :],
                        rcb[dst, :],
                    )

        # ---- final projection: out = outT^T @ Wo + bo
        for m in range(QSH // P):
            ps = psA.tile([P, 512], f32, tag="acc")
            for a in range(AT):
                nc.tensor.matmul(
                    ps[:],
                    r(outT_sb[:, a, m * P:(m + 1) * P]),
                    r(wo_sb[:, a, :]),
                    start=(a == 0),
                    stop=(a == AT - 1),
                )
            ob = outp.tile([P, QD], f32)
            nc.vector.tensor_add(ob[:], ps[:], bo_sb[:])
            nc.sync.dma_start(out_d[m * P:(m + 1) * P, :], ob[:])

    nc.finalize()
    return nc


def _get_nc():
    if "nc" not in _CACHE:
        _CACHE["nc"] = _build_nc()
    return _CACHE["nc"]


def make_in_maps(hidden_states, encoder_hidden_states, Wq, Wk, Wv, Wo, bo):
    hs = np.ascontiguousarray(np.asarray(hidden_states, dtype=np.float32))
    enc = np.ascontiguousarray(np.asarray(encoder_hidden_states, dtype=np.float32))
    wq = np.ascontiguousarray(np.asarray(Wq, dtype=np.float32))
    wk = np.ascontiguousarray(np.asarray(Wk, dtype=np.float32))
    wv = np.ascontiguousarray(np.asarray(Wv, dtype=np.float32))
    wo = np.ascontiguousarray(np.asarray(Wo, dtype=np.float32))
    bo_ = np.ascontiguousarray(np.asarray(bo, dtype=np.float32)).reshape(1, QD)
    encT = [np.ascontiguousarray(enc[b].T) for b in range(B)]
    in_maps = []
    for c in range(NCORES):
        b, s = divmod(c, 4)
        hsT = np.ascontiguousarray(hs[b, s * QSH:(s + 1) * QSH, :].T)
        in_maps.append(
            dict(hsT=hsT, encT=encT[b], wq=wq, wk=wk, wv=wv, wo=wo, bo=bo_)
        )
    return in_maps


def kernel(hidden_states, encoder_hidden_states, Wq, Wk, Wv, Wo, bo):
    global LAST_RESULTS
    from concourse.bass_utils import run_bass_kernel_spmd

    nc = _get_nc()
    in_maps = make_in_maps(
        hidden_states, encoder_hidden_states, Wq, Wk, Wv, Wo, bo
    )
    res = run_bass_kernel_spmd(nc, in_maps, core_ids=list(range(NCORES)))
    LAST_RESULTS = res
    out = np.empty((B, LQ, QD), dtype=np.float32)
    for c in range(NCORES):
        b, s = divmod(c, 4)
        out[b, s * QSH:(s + 1) * QSH, :] = res.results[c]["out"]
    return out



# revision 15
# speedup vs baseline: 1.4988x; 1.4988x over previous
"""CrossAttention kernel for 8 TRN2 NeuronCores.

Reference computation (B=2, Lq=4096, Lkv=1024, query_dim=512, cross_dim=768,
heads=8, dim_head=64, inner=512):
    q = hs @ Wq; k = enc @ Wk; v = enc @ Wv          (per batch)
    attn = softmax(q_h @ k_h^T * scale) per head
    out = concat_h(attn @ v_h) @ Wo + bo

Sharding: 8 cores = 2 batches x 4 query-slices of 1024 queries.  Each core
computes its full slice of the output (all heads), so outputs are disjoint
and no collective is needed.

Per-core dataflow (all matmul operands bf16, PSUM accumulation fp32):
  - host passes hs-slice and encoder transposed (hsT [512,1024], encT
    [768,1024]) pre-cast to bf16, weights bf16
  - qT = Wq^T-weighted hsT   -> [inner=512, q=1024]  (heads along partitions)
  - kT likewise              -> [inner=512, kv=1024]
  - v natural                -> [kv=1024, slots]  slot h = 128 cols holding
        v_h (64) + a ones column + zero padding, arranged so the AV matmul
        output lands partition-aligned with head h's rows of outT and the
        softmax denominator (sum_kv exp) falls out of the same matmul.
  - scoresT_h = k_h qT_h     -> [kv, q] (kv on partitions; head pairs packed
        into the 128x128 PE array via base-partition row tiling)
  - expT = exp(scale * scoresT) on ScalarE, bf16 out (no max-subtraction:
        |scores| small)
  - outT_unnorm_h = v_slot^T @ expT accumulated over kv chunks (PSUM),
        one row of which is the softmax denominator
  - normalize: reciprocal_approx_fast (DVE) + partition_broadcast (GPSIMD)
        + multiply (DVE, bf16 out)
  - the two q-slices (n=0,1) of each head pair are software-pipelined
    together so the tensor engine can run ahead while ScalarE exps drain
  - final = outT^T @ Wo + bo -> [1024, 512] fp32, DMA out
"""

import sys

if "/opt/trn_rl_repo" not in sys.path:
    sys.path.insert(0, "/opt/trn_rl_repo")

import numpy as np

B, LQ, LKV = 2, 4096, 1024
QD, CD = 512, 768
H, DH = 8, 64
INNER = H * DH  # 512
SCALE = DH ** -0.5
NCORES = 8
QSH = LQ // 4  # 1024 queries per core
P = 128

_CACHE: dict = {}
LAST_RESULTS = None  # test harness introspection (exec_time_ns etc.)


def _build_nc():
    from contextlib import ExitStack

    import concourse.bass as bass
    import concourse.tile as tile
    from concourse import bacc, mybir

    f32 = mybir.dt.float32
    bf16 = mybir.dt.bfloat16
    Exp = mybir.ActivationFunctionType.Exp

    nc = bacc.Bacc(trn_type="TRN2")

    hsT_d = nc.declare_dram_parameter("hsT", [QD, QSH], bf16, isOutput=False)
    encT_d = nc.declare_dram_parameter("encT", [CD, LKV], bf16, isOutput=False)
    wq_d = nc.declare_dram_parameter("wq", [QD, INNER], bf16, isOutput=False)
    wk_d = nc.declare_dram_parameter("wk", [CD, INNER], bf16, isOutput=False)
    wv_d = nc.declare_dram_parameter("wv", [CD, INNER], bf16, isOutput=False)
    wo_d = nc.declare_dram_parameter("wo", [INNER, QD], bf16, isOutput=False)
    bo_d = nc.declare_dram_parameter("bo", [1, QD], f32, isOutput=False)
    out_d = nc.declare_dram_parameter("out", [QSH, QD], f32, isOutput=True)

    KC_Q = QD // P   # 4 contraction chunks for q projection
    KC_KV = CD // P  # 6 for k/v projections
    AT = INNER // P  # 4 inner tiles (2 heads each)
    NT = LKV // P    # 8 kv chunks
    QN = QSH // 512  # 2 q slices of 512

    with ExitStack() as ctx:
        tc = ctx.enter_context(tile.TileContext(nc))
        const = ctx.enter_context(tc.tile_pool(name="const", bufs=1))
        acts = ctx.enter_context(tc.tile_pool(name="acts", bufs=1))
        expp = ctx.enter_context(tc.tile_pool(name="expp", bufs=4))
        outp = ctx.enter_context(tc.tile_pool(name="outp", bufs=2))
        small = ctx.enter_context(tc.tile_pool(name="small", bufs=8))
        psA = ctx.enter_context(tc.tile_pool(name="psA", bufs=4, space="PSUM"))
        psS = ctx.enter_context(tc.tile_pool(name="psS", bufs=2, space="PSUM"))

        # ---- input DMA, chunked and spread over both hardware DGE queues
        # (SP + ACT) so the first projection matmuls start as soon as their
        # weight slice + activation chunk have landed.
        wq_sb = const.tile([P, KC_Q, INNER], bf16)
        wqr = wq_d.rearrange("(c p) n -> p c n", p=P)
        for c in range(KC_Q):
            nc.scalar.dma_start(wq_sb[:, c, :], wqr[:, c, :])
        hsT_sb = acts.tile([P, KC_Q, QSH], bf16)
        for c in range(KC_Q):
            nc.sync.dma_start(hsT_sb[:, c, :], hsT_d[c * P:(c + 1) * P, :])
        wk_sb = const.tile([P, KC_KV, INNER], bf16)
        wkr = wk_d.rearrange("(c p) n -> p c n", p=P)
        for c in range(KC_KV):
            nc.scalar.dma_start(wk_sb[:, c, :], wkr[:, c, :])
        encT_sb = acts.tile([P, KC_KV, LKV], bf16)
        for c in range(KC_KV):
            nc.sync.dma_start(encT_sb[:, c, :], encT_d[c * P:(c + 1) * P, :])
        wv_sb = const.tile([P, KC_KV, INNER], bf16)
        nc.scalar.dma_start(wv_sb[:], wv_d.rearrange("(c p) n -> p c n", p=P))
        wo_sb = const.tile([P, AT, QD], bf16)
        nc.scalar.dma_start(wo_sb[:], wo_d.rearrange("(c p) n -> p c n", p=P))
        bo_sb = const.tile([P, QD], f32)
        nc.scalar.dma_start(bo_sb[:], bo_d.ap().to_broadcast((P, QD)))

        # ---- projections
        # qT [inner, q]: lhsT = Wq chunk, rhs = hsT chunk
        qT_sb = acts.tile([P, AT, QSH], bf16)
        for a in range(AT):
            for n in range(QN):
                ps = psA.tile([P, 512], f32, tag="acc")
                for c in range(KC_Q):
                    nc.tensor.matmul(
                        ps[:],
                        wq_sb[:, c, a * P:(a + 1) * P],
                        hsT_sb[:, c, n * 512:(n + 1) * 512],
                        start=(c == 0),
                        stop=(c == KC_Q - 1),
                    )
                nc.vector.tensor_copy(qT_sb[:, a, n * 512:(n + 1) * 512], ps[:])

        # kT [inner, kv]
        kT_sb = acts.tile([P, AT, LKV], bf16)
        for a in range(AT):
            for n in range(LKV // 512):
                ps = psA.tile([P, 512], f32, tag="acc")
                for c in range(KC_KV):
                    nc.tensor.matmul(
                        ps[:],
                        wk_sb[:, c, a * P:(a + 1) * P],
                        encT_sb[:, c, n * 512:(n + 1) * 512],
                        start=(c == 0),
                        stop=(c == KC_KV - 1),
                    )
                nc.vector.tensor_copy(kT_sb[:, a, n * 512:(n + 1) * 512], ps[:])

        # v natural [kv, slots]: every slot h (128 wide) is
        #   [ones (0:64) | v_h (64:128)]
        # so the AV matmul puts the softmax denominator (replicated 64x by
        # the ones columns -- PE cost is per output row, not per column) on
        # psum partitions 0:64, where the custom-DVE reciprocal works, and
        # the head data on partitions 64:128, which the mixed-base DVE mul
        # reads directly (both HW-verified).
        v_sb = acts.tile([P, NT, H * P], bf16)
        vv4 = v_sb.rearrange("p t (s c) -> p t s c", c=P)
        for t in range(NT):
            nc.vector.memset(vv4[:, t, :, 0:64], 1.0)
        for t in range(NT):
            ps = psA.tile([P, 512], f32, tag="acc")
            for c in range(KC_KV):
                nc.tensor.matmul(
                    ps[:],
                    encT_sb[:, c, t * P:(t + 1) * P],
                    wv_sb[:, c, :],
                    start=(c == 0),
                    stop=(c == KC_KV - 1),
                )
            vv = v_sb.rearrange("p t (s c) -> p t s c", c=P)
            pv = ps.rearrange("p (s c) -> p s c", c=DH)
            nc.vector.tensor_copy(vv[:, t, :, DH:P], pv[:])

        # ---- attention + output accumulation
        # q-slice (n) outer so the final projection of n=0 can overlap the
        # attention of n=1.  Within an n, head-pair blocks run two at a time
        # as interleaved pipelines: tensor work of one fills the ScalarE exp
        # latency of the other.  PSUM: 4 x av [128,512] (1 bank each) +
        # 2 x ss [128,1024] (2 banks each) = 8 banks.
        # one outT tile per q-slice so the final projection of slice n only
        # depends on that slice's normalize writes (tile-granular deps)
        outT_n = [
            acts.tile([P, AT, 512], bf16, name=f"outT{n}") for n in range(QN)
        ]

        def final_proj(m):
            n = m // 4
            mm = m % 4
            ps = psA.tile([P, 512], f32, tag="acc", name=f"fp{m}")
            for a in range(AT):
                nc.tensor.matmul(
                    ps[:],
                    outT_n[n][:, a, mm * P:(mm + 1) * P],
                    wo_sb[:, a, :],
                    start=(a == 0),
                    stop=(a == AT - 1),
                )
            ob = outp.tile([P, QD], f32, name=f"ob{m}")
            nc.vector.tensor_add(ob[:], ps[:], bo_sb[:])
            nc.sync.dma_start(out_d[m * P:(m + 1) * P, :], ob[:])

        for n in range(QN):
            for hpp in range(AT // 2):
                # n=0's final projection is emitted between n=1's two block
                # sections: by then n=0's normalizes have long drained, so
                # the tensor engine never stalls on them, and only n=1's
                # m-tiles remain for the tail.
                if n == 1 and hpp == 1:
                    for m in range(4):
                        final_proj(m)
                hps = (2 * hpp, 2 * hpp + 1)
                av = {}
                ex = {}
                for hp in hps:
                    av[hp] = (
                        psA.tile([P, 512], f32, tag="acc", name=f"av{n}_{hp}_0"),
                        psA.tile([P, 512], f32, tag="acc", name=f"av{n}_{hp}_1"),
                    )
                for t in range(NT):
                    for hp in hps:
                        ss = psS.tile([P, 1024], f32)
                        for i in range(2):
                            pr = slice(i * 64, (i + 1) * 64)
                            nc.tensor.matmul(
                                ss[:, i * 512:(i + 1) * 512],
                                kT_sb[pr, hp, t * P:(t + 1) * P],
                                qT_sb[pr, hp, n * 512:(n + 1) * 512],
                                start=True,
                                stop=True,
                            )
                        e = expp.tile([P, 1024], bf16)
                        nc.scalar.activation(e[:], ss[:], Exp, scale=SCALE)
                        ex[hp] = e
                    for hp in hps:
                        for i in range(2):
                            s = 2 * hp + i
                            nc.tensor.matmul(
                                av[hp][i][:],
                                v_sb[:, t, s * P:(s + 1) * P],
                                ex[hp][:, i * 512:(i + 1) * 512],
                                start=(t == 0),
                                stop=(t == NT - 1),
                            )
                # normalize: denom (replicated, parts 0:64 where the custom
                # DVE recip works) -> recip -> mixed-base mul with the data
                # half (parts 64:128), writing outT rows for this head.
                for hp in hps:
                    for i in range(2):
                        dst = slice(0, 64) if i == 0 else slice(64, 128)
                        rcb = small.tile([P, 512], f32, tag="rcb")
                        nc.vector.reciprocal_approx_fast(
                            rcb[0:64, :], av[hp][i][0:64, :]
                        )
                        nc.vector.tensor_mul(
                            outT_n[n][dst, hp, :],
                            av[hp][i][64:128, :],
                            rcb[0:64, :],
                        )

        for m in range(4, 8):
            final_proj(m)

    nc.finalize()
    return nc


def _get_nc():
    if "nc" not in _CACHE:
        _CACHE["nc"] = _build_nc()
    return _CACHE["nc"]


def make_in_maps(hidden_states, encoder_hidden_states, Wq, Wk, Wv, Wo, bo):
    import ml_dtypes

    bf16 = ml_dtypes.bfloat16
    hs = np.asarray(hidden_states, dtype=np.float32)
    enc = np.asarray(encoder_hidden_states, dtype=np.float32)
    wq = np.ascontiguousarray(np.asarray(Wq, dtype=np.float32)).astype(bf16)
    wk = np.ascontiguousarray(np.asarray(Wk, dtype=np.float32)).astype(bf16)
    wv = np.ascontiguousarray(np.asarray(Wv, dtype=np.float32)).astype(bf16)
    wo = np.ascontiguousarray(np.asarray(Wo, dtype=np.float32)).astype(bf16)
    bo_ = np.ascontiguousarray(np.asarray(bo, dtype=np.float32)).reshape(1, QD)
    encT = [np.ascontiguousarray(enc[b].T).astype(bf16) for b in range(B)]
    in_maps = []
    for c in range(NCORES):
        b, s = divmod(c, 4)
        hsT = np.ascontiguousarray(hs[b, s * QSH:(s + 1) * QSH, :].T).astype(bf16)
        in_maps.append(
            dict(hsT=hsT, encT=encT[b], wq=wq, wk=wk, wv=wv, wo=wo, bo=bo_)
        )
    return in_maps


def kernel(hidden_states, encoder_hidden_states, Wq, Wk, Wv, Wo, bo):
    global LAST_RESULTS
    from concourse.bass_utils import run_bass_kernel_spmd

    nc = _get_nc()
    in_maps = make_in_maps(
        hidden_states, encoder_hidden_states, Wq, Wk, Wv, Wo, bo
    )
    res = run_bass_kernel_spmd(nc, in_maps, core_ids=list(range(NCORES)))
    LAST_RESULTS = res
    out = np.empty((B, LQ, QD), dtype=np.float32)
    for c in range(NCORES):
        b, s = divmod(c, 4)
        out[b, s * QSH:(s + 1) * QSH, :] = res.results[c]["out"]
    return out
